# revision 10
# baseline (speedup 1.0000x reference)
"""Segment-sum (scatter-add) kernel for Trainium2, 8 NeuronCores. v5.

out[n, :] = sum_{e : index[e] == n} input[e, :]   (N=50000 segments, d=64)

Host side (data movement / re-encoding only, no arithmetic reduction):
  1. argsort(index) -> edges grouped by destination segment; error-
     feedback quantize rows to fp8 e4m3 in segment order (rounding
     residual carried to the same segment's next edge, so the device's
     fp32 sum of quantized rows is exact to ~one final rounding).
  2. Sort segments by count (desc).  Super-chunks of 256 consecutive
     sorted segments (32 per core x 8 cores) share a slot capacity
     khat = ceil(max_count/4)*4.  Every core gets the same khat
     sequence -> identical SPMD program; fill ~95%.
  3. A chunk holds 32 segments x khat slots = khat/4 tiles of 128
     slots.  Slot p of tile t belongs to chunk row (128t+p)//khat --
     a CONSTANT one-hot matrix per (khat, t) shared by all chunks of
     that class (~83 distinct [128, 32] fp8 matrices, DMA'd once).
  4. Chunks are grouped 4-at-a-time (same class) into "cgroups"; the
     edge tiles are laid out block-interleaved so one DoubleRow matmul
     processes tile pair (2u, 2u+1) of all 4 chunks at once.

Device side (all FLOPs):
  Per cgroup: T/2 fp8 DoubleRow matmuls, each with constant one-hot
  lhsT [128, 2, 32] and rhs [128, 2, 256] (out psum[32, 256], the
  weight load is amortized over 8 edge tiles).  Flush: VectorE copies
  16-chunk groups PSUM->SBUF (fp32 -> fp16); the Scalar queue streams
  them to HBM.

Host finalization: place per-chunk row blocks into the [50000, 64]
output (pure scatter placement, each segment lives in exactly one
chunk row).
"""

import os
import sys

for _p in ("/opt/trn_rl_repo", "/opt/pypackages"):
    if _p not in sys.path:
        sys.path.append(_p)

import numpy as np
import ml_dtypes

import concourse.mybir as mybir
from concourse import bacc
from concourse.tile import TileContext
from concourse.bass_utils import run_bass_kernel_spmd

N_CORES = 8
P = 128               # partitions / contraction dim per tile
D = 64                # feature dim
ROWS = 32             # segments per chunk (psum partition dim)
GROUP = ROWS * N_CORES  # segments per super-chunk
CG = 4                # chunks per matmul batch (rhs free = 2*CG*64 = 512)
PSUM_CG = 4           # cgroups per PSUM tile ([32, 4*256] f32 = 2 banks)
STRIP_TILES = 256     # tile budget per input DMA strip (16KB/partition)

F32 = mybir.dt.float32
F16 = mybir.dt.float16
F8 = mybir.dt.float8e4
NP_F8 = ml_dtypes.float8_e4m3


def ef_quantize(x_sorted, idx_sorted, n_segments):
    """Error-feedback rounding to e4m3 along each segment's edge run."""
    n = len(idx_sorted)
    counts = np.bincount(idx_sorted, minlength=n_segments)
    starts = np.zeros(n_segments, dtype=np.int64)
    starts[1:] = np.cumsum(counts)[:-1]
    pos = np.arange(n, dtype=np.int64) - starts[idx_sorted]
    maxc = int(counts.max()) if n else 0

    xq = np.empty((n, D), dtype=NP_F8)
    carry = np.zeros((n_segments, D), dtype=np.float32)
    for k in range(maxc):
        sel = np.flatnonzero(pos == k)
        if len(sel) == 0:
            break
        segs = idx_sorted[sel]
        v = x_sorted[sel] + carry[segs]
        q = v.astype(NP_F8)
        carry[segs] = v - q.astype(np.float32)
        xq[sel] = q
    return xq


def plan(counts, n_segments):
    """Chunk classes and cgroup layout, identical for every core."""
    seg_order = np.argsort(-counts, kind="stable")
    n_groups = -(-n_segments // GROUP)
    pad_segs = n_groups * GROUP - n_segments

    cs = counts[seg_order]
    cs_p = np.concatenate([cs, np.zeros(pad_segs, dtype=cs.dtype)])
    km = cs_p.reshape(n_groups, GROUP).max(1)
    khat = np.maximum((km + 3) // 4 * 4, 4).astype(np.int64)
    assert khat.max() <= P

    # cgroups: runs of <=CG consecutive chunks of the same class
    cgroups = []  # (cg0, n_c, k, T)
    i = 0
    while i < n_groups:
        k = int(khat[i])
        j = i
        while j < n_groups and j - i < CG and int(khat[j]) == k:
            j += 1
        cgroups.append((i, j - i, k, k // 4))
        i = j

    # slot base of each cgroup / chunk block layout
    cg_base = []
    chunk_of = np.zeros(n_groups, dtype=np.int64)  # chunk -> cgroup idx
    base = 0
    for gi, (cg0, n_c, k, T) in enumerate(cgroups):
        cg_base.append(base)
        for c in range(cg0, cg0 + n_c):
            chunk_of[c] = gi
        base += n_c * ROWS * k  # == n_c * T * P
    slots_core = base
    assert slots_core % P == 0
    return seg_order, khat, cgroups, cg_base, chunk_of, slots_core


def build_device_arrays(input_np, index_np, n_segments):
    input_np = np.asarray(input_np, dtype=np.float32).reshape(-1, D)
    index_np = np.asarray(index_np).astype(np.int64, copy=False).ravel()
    n_edges = input_np.shape[0]

    counts = np.bincount(index_np, minlength=n_segments)
    seg_order, khat, cgroups, cg_base, chunk_of, slots_core = plan(
        counts, n_segments
    )
    n_groups = len(khat)
    tiles_core = slots_core // P

    cg_base = np.asarray(cg_base, dtype=np.int64)
    cg_nc = np.array([g[1] for g in cgroups], dtype=np.int64)
    cg_c0 = np.array([g[0] for g in cgroups], dtype=np.int64)

    # seg (sorted position) -> (chunk i, core j, row r)
    pos_all = np.arange(n_segments, dtype=np.int64)
    gi = pos_all // GROUP
    within = pos_all % GROUP
    core = within // ROWS
    row = within % ROWS
    # chunk-linear slot s = row*khat + pos; global slot =
    #   core*slots_core + cg_base[cg] + (s>>7)*(n_c*128) + c_off*128 + (s&127)
    cgi = chunk_of[gi]
    seg_core = core
    seg_chunk = gi
    # per-seg constants for the edge scatter
    seg_k = khat[gi]
    seg_cgbase = cg_base[cgi]
    seg_nc = cg_nc[cgi]
    seg_coff = gi - cg_c0[cgi]

    by_seg = np.empty((n_segments, 5), dtype=np.int64)
    sid = seg_order
    by_seg[sid, 0] = seg_core * slots_core + seg_cgbase
    by_seg[sid, 1] = seg_nc
    by_seg[sid, 2] = seg_coff
    by_seg[sid, 3] = row * seg_k  # chunk-linear start
    by_seg[sid, 4] = 0

    order = np.argsort(index_np, kind="stable")
    idx_sorted = index_np[order]
    x_sorted = input_np[order]
    xq_sorted = ef_quantize(x_sorted, idx_sorted, n_segments)

    starts = np.zeros(n_segments, dtype=np.int64)
    starts[1:] = np.cumsum(counts)[:-1]
    epos = np.arange(n_edges, dtype=np.int64) - starts[idx_sorted]
    s_lin = by_seg[idx_sorted, 3] + epos  # chunk-linear slot
    slot = (
        by_seg[idx_sorted, 0]
        + (s_lin >> 7) * (by_seg[idx_sorted, 1] * P)
        + by_seg[idx_sorted, 2] * P
        + (s_lin & (P - 1))
    )

    X_all = np.zeros((N_CORES * slots_core, D), dtype=NP_F8)
    X_all[slot] = xq_sorted

    # one-hot constant library
    ks = sorted(set(int(k) for k in khat))
    mat_off = {}
    nmat = 0
    for k in ks:
        mat_off[k] = nmat
        nmat += k // 4
    OHC = np.zeros((P, nmat * ROWS), dtype=NP_F8)
    pp = np.arange(P)
    for k in ks:
        for t in range(k // 4):
            jj = (P * t + pp) // k
            m = mat_off[k] + t
            OHC[pp, m * ROWS + jj] = 1.0

    in_maps = []
    for c in range(N_CORES):
        xt = X_all[c * slots_core : (c + 1) * slots_core].reshape(
            tiles_core, P, D
        )
        xc = xt.transpose(1, 0, 2).reshape(P, tiles_core * D)
        in_maps.append({"x": np.ascontiguousarray(xc), "ohc": OHC})

    # chunk i lives at OUT block column chunk_of[i]*CG + (i - cg_c0) --
    # every cgroup owns a fixed CG*D-wide slot (psum bank alignment)
    subcol = chunk_of * CG + (np.arange(n_groups) - cg_c0[chunk_of])
    n_cg = len(cgroups)

    def assemble(core_outs):
        out = np.zeros((n_segments, D), dtype=np.float32)
        for j, o in enumerate(core_outs):
            o3 = np.asarray(o, dtype=np.float32).reshape(ROWS, n_cg * CG, D)
            rows = (
                o3[:, subcol, :]
                .transpose(1, 0, 2)
                .reshape(n_groups * ROWS, D)
            )
            pos = np.arange(n_groups) * GROUP + ROWS * j
            segpos = (pos[:, None] + np.arange(ROWS)[None, :]).ravel()
            valid = segpos < n_segments
            out[seg_order[segpos[valid]]] = rows[valid]
        return out

    return khat, cgroups, mat_off, tiles_core, in_maps, assemble


def build_bass(khat, cgroups, mat_off, n_tiles):
    nc = bacc.Bacc(
        "TRN2", target_bir_lowering=False, debug=False, num_devices=N_CORES
    )
    n_chunks = len(khat)
    nmat = mat_off[max(mat_off)] + max(mat_off) // 4

    n_cg = len(cgroups)
    X = nc.dram_tensor("x", [P, n_tiles * D], F8, kind="ExternalInput")
    OHCD = nc.dram_tensor("ohc", [P, nmat * ROWS], F8, kind="ExternalInput")
    OUT = nc.dram_tensor(
        "out", [ROWS, n_cg * CG * D], F16, kind="ExternalOutput"
    )

    # cgroup tile bases
    cg_tbase = [0]
    for (cg0, n_c, k, T) in cgroups:
        cg_tbase.append(cg_tbase[-1] + n_c * T)
    assert cg_tbase[-1] == n_tiles

    # input DMA strips: whole cgroups, ramped tile budgets
    ramp = tuple(
        int(v) for v in os.environ.get("RAMP", "64,128").split(",") if v
    )
    strips = []  # (cg_lo, cg_hi, tile_lo, strip_tiles)
    ci = 0
    budgets = list(ramp)
    while ci < n_cg:
        budget = budgets.pop(0) if budgets else STRIP_TILES
        lo = ci
        st = 0
        while ci < n_cg and (
            st == 0 or st + cgroups[ci][1] * cgroups[ci][3] <= budget
        ):
            st += cgroups[ci][1] * cgroups[ci][3]
            ci += 1
        strips.append((lo, ci, cg_tbase[lo], st))
    # ramp the tail down so the post-DMA compute drain is short
    while int(os.environ.get("TAILSPLIT", "0")) and strips and strips[-1][3] > 96 and strips[-1][1] - strips[-1][0] > 1:
        lo, hi, tlo, st = strips.pop()
        mid = lo
        acc = 0
        while mid < hi - 1 and acc < st * 2 // 3:
            acc += cgroups[mid][1] * cgroups[mid][3]
            mid += 1
        strips.append((lo, mid, tlo, acc))
        strips.append((mid, hi, cg_tbase[mid], st - acc))
        if st - acc <= 96:
            break
    max_strip_tiles = max(s[3] for s in strips)

    cg_strip = {}
    for si, (lo, hi, tlo, st) in enumerate(strips):
        for g in range(lo, hi):
            cg_strip[g] = si

    with TileContext(nc) as tc:
        with (
            tc.tile_pool(name="const", bufs=1) as cpool,
            tc.tile_pool(name="xin", bufs=5) as xpool,
            tc.tile_pool(name="acc", bufs=4, space="PSUM") as ppool,
            tc.tile_pool(name="outp", bufs=4) as opool,
        ):
            ohc_t = cpool.tile([P, nmat * ROWS], F8)
            nc.gpsimd.dma_start(out=ohc_t[:], in_=OHCD[:, :])
            xs_tiles = {}
            for si, (lo, hi, tlo, st) in enumerate(strips):
                xs = xpool.tile([P, max_strip_tiles * D], F8, tag="xs")
                nc.sync.dma_start(
                    out=xs[:, : st * D],
                    in_=X[:, tlo * D : (tlo + st) * D],
                )
                xs_tiles[si] = xs

            gi = 0
            while gi < n_cg:
                # one psum tile covers up to PSUM_CG cgroups; each cgroup
                # owns a fixed CG*D slot so no accumulation region ever
                # crosses a PSUM bank boundary
                gn = min(PSUM_CG, n_cg - gi)
                width = gn * CG * D
                ps = ppool.tile([ROWS, PSUM_CG * CG * D], F32, tag="ps")
                # NOTE: all matmuls of one cgroup must stay consecutive --
                # `start` lazily zeroes a whole 2KB PSUM bank, so
                # interleaving accumulations that share a bank corrupts
                # the neighbor's partial sums.
                for g in range(gi, gi + gn):
                    cg0, n_c, k, T = cgroups[g]
                    si = cg_strip[g]
                    xs = xs_tiles[si]
                    off = (cg_tbase[g] - strips[si][2]) * D
                    bs = n_c * D  # elems per tile-block
                    m0 = mat_off[k]
                    col = (g - gi) * CG * D
                    pso = ps[:, col : col + n_c * D]
                    npair = T // 2
                    for u in range(npair):
                        nc.tensor.matmul(
                            pso,
                            lhsT=ohc_t[
                                :, (m0 + 2 * u) * ROWS : (m0 + 2 * u + 2) * ROWS
                            ].rearrange("p (t g) -> p t g", t=2, g=ROWS),
                            rhs=xs[
                                :, off + 2 * u * bs : off + (2 * u + 2) * bs
                            ].rearrange("p (t d) -> p t d", t=2, d=bs),
                            start=(u == 0),
                            stop=(u == npair - 1 and T % 2 == 0),
                            perf_mode=mybir.MatmulPerfMode.DoubleRow,
                        )
                    if T % 2 == 1:
                        nc.tensor.matmul(
                            pso,
                            lhsT=ohc_t[:, (m0 + T - 1) * ROWS : (m0 + T) * ROWS],
                            rhs=xs[:, off + (T - 1) * bs : off + T * bs],
                            start=(T == 1),
                            stop=True,
                        )
                ost = opool.tile([ROWS, PSUM_CG * CG * D], F16, tag="ost")
                if (gi // PSUM_CG) % 2:
                    nc.scalar.copy(ost[:, :width], ps[:, :width])
                else:
                    nc.vector.tensor_copy(ost[:, :width], ps[:, :width])
                nc.scalar.dma_start(
                    out=OUT[:, gi * CG * D : gi * CG * D + width],
                    in_=ost[:, :width],
                )
                gi += gn
    nc.compile()
    return nc


def _run(input_np, index_np, n_segments, trace=False, trace_kwargs=None):
    khat, cgroups, mat_off, tiles_core, in_maps, assemble = (
        build_device_arrays(input_np, index_np, n_segments)
    )
    nc = build_bass(khat, cgroups, mat_off, tiles_core)
    res = run_bass_kernel_spmd(
        nc,
        in_maps,
        core_ids=list(range(N_CORES)),
        trace=trace,
        **(trace_kwargs or {}),
    )
    outs = [np.asarray(r["out"]) for r in res.results]
    return assemble(outs), res


def kernel(input, index):
    # one retry: a transient device fault surfaces as NaN/Inf or a
    # runtime error; the kernel itself is deterministic
    last = None
    for attempt in range(2):
        try:
            out, _ = _run(np.asarray(input), np.asarray(index), 50000)
        except Exception:
            if attempt:
                raise
            continue
        last = out
        if np.isfinite(out).all():
            return out
    return last
